# revision 15
# baseline (speedup 1.0000x reference)
"""Trainium2 kernel for nn_DynamicGeometricRotation — 3 collective-free
SPMD launches (collectives cost ~185us fixed in this environment, so the
params exchange bounces through host DRAM instead).

Reference (B=16, S=8192, D=128, H=512, R=3):
    pooled = x.mean(S); h = gelu(pooled @ W1.T + b1)
    params = (h @ W2.T + b2) -> [B, R, D, D]; G_i = 0.5(P_i - P_i^T)
    out = x @ expm(G_0) @ expm(G_1) @ expm(G_2)

Key idea: the device only ever streams x in fp8 and returns the fp8
DELTA  d = x8 @ (R - I)  (scaled x16); the host computes y = x + d/16
against the exact fp32 x. Because ||R - I|| ~ 0.3, fp8 quantization of
both the x stream and the delta stream contributes only ~1e-2 relative
error (gate is 2e-2), while halving rot's HBM traffic vs bf16 in/out.

  L1 "pool"   batch-sharded sum over S. x arrives in natural-collated
              fp8 tiles xn8[b,c,sp,t,d] (s = 2048c + 128t + sp); PE
              reduces via ones-STATIONARY DoubleRow fp8 matmuls (x is
              the moving operand, 2 s-tiles per pass), PSUM-accumulated
              -> pooled sums [BPC, D] f32.
  host        concat pooled, pack [pallT | W1T/S | b1] into one f32
              tensor (no math).
  L2 "params" G is antisymmetric: device computes only the 24384
              independent entries. W2u = 0.5(W2^T - swap) restricted to
              strict-upper entries, fp8 x64, column-sharded (1.5MB/
              core). On-device MLP1 (f32, ACT exact-erf Gelu only ACT
              function -> no act-table thrash) + 6 panel matmuls
              (bf16 hT stationary, fp8 W2u moving, DVE psum copies)
              -> [16, 3072] bf16.
  host        scatter upper entries, mirror with negation, add b2 skew
              bias (permutation + negation only).
  L3 "rot"    batch-sharded. x streams as fp8 transposed tiles
              xq8[b,c,d,j] (s = 2048c + j). Degree-4 Taylor expm +
              rotation chain -> Rm = 16(R - I) bf16, loaded STATIONARY
              (2 Ldweights total); x is the moving operand. PSUM -> fp8
              copies alternate ACT/DVE; delta out on the SP queue.
"""

import contextlib
import math

import numpy as np

import concourse.bass as bass
import concourse.mybir as mybir
import concourse.tile as tile
from concourse.bass_utils import run_bass_kernel_spmd
from concourse.masks import make_identity

F32 = mybir.dt.float32
BF16 = mybir.dt.bfloat16
F8 = mybir.dt.float8e4

B, S, D = 16, 8192, 128
H = 512
NROT = 3
NCORES = 8
BPC = B // NCORES             # 2 batches per core
NCH = 4                       # chunks per batch
CW = S // NCH                 # 2048 columns per chunk
TPC = 16                      # s-tiles per chunk (128 each)
KT = H // 128                 # 4 k-tiles
NU = D * (D - 1) // 2         # 8128 strict-upper entries per rotation
JPCU = 3072                   # padded upper-entry columns per core
NJ = 512                      # W2 panel width
NPAN = JPCU // NJ             # 6 panels
DSC = 16.0                    # delta output scale


def _split_sync_waits(nc, max_waits=1):
    """walrus rejects >1 semaphore wait per instruction; split extras into
    preceding same-engine NOPs (engine stalls there, preserving order)."""
    for fn in nc.m.functions:
        for bb in fn.blocks:
            insts = bb.instructions
            i = 0
            while i < len(insts):
                inst = insts[i]
                si = inst.sync_info
                if si is not None and len(si.on_wait) > max_waits:
                    waits = list(si.on_wait)
                    keep = waits[-max_waits:]
                    rest = waits[:-max_waits]
                    nops = []
                    for j in range(0, len(rest), max_waits):
                        nops.append(
                            mybir.InstNoOp(
                                name=f"{inst.name}-waitsplit-{j}",
                                engine=inst.engine,
                                sync_info=mybir.SyncInfo(
                                    on_wait=rest[j : j + max_waits], on_update=[]
                                ),
                                bass_nofuse=True,
                            )
                        )
                    inst.sync_info = mybir.SyncInfo(
                        on_wait=keep, on_update=list(si.on_update)
                    )
                    for k, nop in enumerate(nops):
                        insts.insert(i + k, nop)
                    i += len(nops)
                i += 1
    return nc


def _elide_ldweights(nc):
    """Remove back-to-back InstLdweights that reload the exact same
    weights (the PE array retains stationary weights between matmuls).
    Sync info from an elided load is merged onto the next PE instruction
    (its paired matmul)."""
    for fn in nc.m.functions:
        for bb in fn.blocks:
            insts = bb.instructions
            sig = None
            drop = []
            for idx, inst in enumerate(insts):
                if not isinstance(inst, mybir.InstLdweights):
                    continue
                s = (
                    str(inst.ins[0]), str(inst.perf_mode),
                    str(inst.is_transpose), str(inst.tile_position),
                    str(inst.tile_size),
                )
                if s != sig:
                    sig = s
                    continue
                si = inst.sync_info
                if si is not None and (si.on_wait or si.on_update):
                    # move the load's sync onto the next PE instruction
                    nxt = None
                    for j in range(idx + 1, len(insts)):
                        if insts[j].engine == mybir.EngineType.PE:
                            nxt = insts[j]
                            break
                    if nxt is None:
                        sig = s
                        continue
                    nsi = nxt.sync_info
                    nw = list(si.on_wait) + (list(nsi.on_wait) if nsi else [])
                    nu = (list(nsi.on_update) if nsi else []) + list(si.on_update)
                    nxt.sync_info = mybir.SyncInfo(on_wait=nw, on_update=nu)
                drop.append(idx)
            for idx in reversed(drop):
                del insts[idx]
    return nc


def _dp(nc, name, shape, is_out, io_internal, dtype=F32):
    if io_internal:
        return nc.dram_tensor(name, shape, dtype)
    return nc.declare_dram_parameter(name, shape, dtype, isOutput=is_out)


def _bench_io(nc, io_internal):
    if not io_internal:
        return
    dummy = nc.declare_dram_parameter("bench_dummy", [1, 1], F32, isOutput=False)
    sink = nc.declare_dram_parameter("bench_sink", [1, 1], F32, isOutput=True)
    with nc.Block() as blk, nc.semaphore("bench_dsem") as dsem:
        @blk.gpsimd
        def _(gp):
            gp.dma_start(out=sink[:, :], in_=dummy[:, :]).then_inc(dsem, 16)
            gp.wait_ge(dsem, 16)


def _maybe_repeat(tc, nc, repeat):
    if repeat == 1:
        return contextlib.nullcontext()
    E = mybir.EngineType
    return tc.For_i(0, repeat, hint_engines=(E.PE, E.DVE, E.Activation, E.SP, E.Pool))


def build_pool(repeat=1, io_internal=False, split=True):
    """xn8 natural-collated fp8 -> pooled [BPC, D] f32 (sums over S).

    ones [128, 2, 1] is the stationary operand (one Ldweights ever); x
    tiles stream through the PE as the moving operand in fp8 DoubleRow
    mode (2 s-tiles per matmul), accumulating each batch's sum in PSUM.
    The two batches ride the SP and ACT DMA queues in parallel.
    """
    nc = bass.Bass(target_bir_lowering=False)
    xn = _dp(nc, "xn8", [BPC, NCH, 128, TPC, D], False, io_internal, F8)
    out = _dp(nc, "pooled", [BPC, D], True, io_internal)
    PM = mybir.MatmulPerfMode
    with tile.TileContext(nc) as tc:
        with (
            tc.tile_pool(name="const", bufs=1) as cpool,
            tc.tile_pool(name="xin", bufs=10) as xpool,
            tc.tile_pool(name="ps", bufs=2, space="PSUM") as psP,
        ):
            ones2 = cpool.tile([128, 2, 128], F8, name="ones2", tag="ones2")
            nc.vector.memset(ones2, 1.0)
            with _maybe_repeat(tc, nc, repeat):
                pps = [
                    psP.tile([128, D], F32, tag=f"pps{b}", name=f"pps{b}")
                    for b in range(BPC)
                ]
                for c in range(NCH):
                    for b in range(BPC):
                        xt = xpool.tile([128, TPC, D], F8, tag=f"xt{b}")
                        q = nc.sync if b == 0 else nc.scalar
                        q.dma_start(out=xt, in_=xn[b, c])
                        for t2 in range(TPC // 2):
                            nc.tensor.matmul(
                                pps[b],
                                lhsT=ones2,
                                rhs=xt[:, 2 * t2 : 2 * t2 + 2, :],
                                start=(c == 0 and t2 == 0),
                                stop=(c == NCH - 1 and t2 == TPC // 2 - 1),
                                perf_mode=PM.DoubleRow,
                            )
                pool_sb = cpool.tile([1, BPC, D], F32, tag="pool_sb")
                for b in range(BPC):
                    nc.scalar.copy(pool_sb[:, b, :], pps[b][0:1, :])
                nc.sync.dma_start(
                    out=out.rearrange("b d -> (b d)"),
                    in_=pool_sb.rearrange("p b d -> p (b d)"),
                )
    _bench_io(nc, io_internal)
    return _split_sync_waits(_elide_ldweights(nc)) if split else nc


def build_params(repeat=1, io_internal=False, split=True):
    """sin [128, 16+512+4] f32 (pallT | W1T/S | b1) + W2u panels ->
    upper-entry shard [B, JPCU] bf16."""
    nc = bass.Bass(target_bir_lowering=False)
    sin_d = _dp(nc, "sin", [D, B + H + KT], False, io_internal)
    w2u = _dp(nc, "w2u", [NPAN, D, KT * NJ], False, io_internal, F8)
    out = _dp(nc, "params", [B, JPCU], True, io_internal, BF16)
    AF = mybir.ActivationFunctionType
    with tile.TileContext(nc) as tc:
        with (
            tc.tile_pool(name="const", bufs=2) as cpool,
            tc.tile_pool(name="w", bufs=2) as wpool,
            tc.tile_pool(name="ps", bufs=3, space="PSUM") as psMM,
        ):
            warm = cpool.tile([128, 128], BF16, name="warm", tag="warm")
            nc.vector.memset(warm, 0.0)
            with _maybe_repeat(tc, nc, repeat):
                sin_sb = cpool.tile([128, B + H + KT], F32, tag="sin_sb")
                nc.scalar.dma_start(out=sin_sb, in_=sin_d[:, :])
                wtiles = []
                for jo in range(NPAN):
                    w = wpool.tile([128, KT, NJ], F8, name=f"w{jo}", tag=f"w{jo}")
                    nc.sync.dma_start(
                        out=w, in_=w2u[jo].rearrange("p (kt j) -> p kt j", kt=KT)
                    )
                    wtiles.append(w)
                # no-wait warm-up matmuls keep PE busy from t~0 so the
                # p-state ramp covers the panel matmuls (gaps reset it)
                for _ in range(8):
                    wp = psMM.tile([128, NJ], F32, tag="pp")
                    nc.tensor.matmul(wp[:, 0:128], lhsT=warm, rhs=warm,
                                     start=True, stop=True)
                hT = cpool.tile([128, KT, B], BF16, tag="hT")
                for k in range(KT):
                    mp = psMM.tile([128, NJ], F32, tag="pp")
                    nc.tensor.matmul(
                        mp[:, 0:B],
                        lhsT=sin_sb[:, B + k * 128 : B + (k + 1) * 128],
                        rhs=sin_sb[:, 0:B],
                        start=True, stop=True,
                    )
                    nc.scalar.activation(
                        hT[:, k, :], mp[:, 0:B], AF.Gelu,
                        bias=sin_sb[:, B + H + k : B + H + k + 1], scale=1.0,
                    )
                for _ in range(4):
                    wp = psMM.tile([128, NJ], F32, tag="pp")
                    nc.tensor.matmul(wp[:, 0:128], lhsT=warm, rhs=warm,
                                     start=True, stop=True)
                params_sb = cpool.tile([B, JPCU], BF16, tag="params_sb")
                for jo in range(NPAN):
                    pp = psMM.tile([128, NJ], F32, tag="pp")
                    for k in range(KT):
                        nc.tensor.matmul(
                            pp[0:B, :],
                            lhsT=hT[:, k, :],
                            rhs=wtiles[jo][:, k, :],
                            start=(k == 0),
                            stop=(k == KT - 1),
                        )
                    nc.vector.tensor_scalar_mul(
                        params_sb[:, jo * NJ : (jo + 1) * NJ], pp[0:B, :], 1.0 / 64.0
                    )
                nc.scalar.dma_start(out=out[:, :], in_=params_sb)
    _bench_io(nc, io_internal)
    return _split_sync_waits(_elide_ldweights(nc)) if split else nc


def build_rot(repeat=1, io_internal=False, split=True):
    """xq8 [BPC, NCH, D, CW] fp8 + biased G [D, 2*NROT, D] bf16 ->
    dq [BPC, D, NCH, CW] fp8 = 16 * (x8 @ (R - I))^T tiles.

    expm by degree-4 Taylor, rebuilt around PE accumulation and G's
    antisymmetry (mm(g,g) = -G^2, mm(g,I) = -G, identity matmuls add
    I): r = I + G + (-G^2)@(-tb) accumulates in PSUM with only two DVE
    slab ops (ihg, tbneg) on the serial chain. Rm = 16(R - I) is the
    stationary einsum operand; fp8 x tiles stream through the PE. PSUM
    -> fp8 copies alternate ACT/DVE; pair outs alternate SP/ACT queues.
    """
    nc = bass.Bass(target_bir_lowering=False)
    x = _dp(nc, "xq8", [BPC, NCH, D, CW], False, io_internal, F8)
    g_d = _dp(nc, "g", [D, 2 * NROT, D], False, io_internal, BF16)
    dq = _dp(nc, "dq", [BPC, D, NCH, CW], True, io_internal, F8)
    A = mybir.AluOpType
    AF = mybir.ActivationFunctionType
    with tile.TileContext(nc) as tc:
        with (
            tc.tile_pool(name="const", bufs=1) as cpool,
            tc.tile_pool(name="xin", bufs=10) as xpool,
            tc.tile_pool(name="gex", bufs=2) as gpool,
            tc.tile_pool(name="chain", bufs=2) as chpool,
            tc.tile_pool(name="yout", bufs=3) as ypool,
            tc.tile_pool(name="psE", bufs=1, space="PSUM") as psE,
            tc.tile_pool(name="psJ", bufs=1, space="PSUM") as psJ,
            tc.tile_pool(name="psY", bufs=2, space="PSUM") as psY,
        ):
            warm = cpool.tile([128, 128], BF16, name="warm")
            nc.vector.memset(warm, 0.0)
            ident_bf = cpool.tile([128, 128], BF16)
            make_identity(nc, ident_bf)
            ident16 = cpool.tile([128, 128], BF16)
            nc.vector.tensor_scalar_mul(ident16, ident_bf, DSC)
            ident6_h = cpool.tile([128, 2 * NROT, 128], BF16)
            for i in range(2 * NROT):
                nc.vector.tensor_scalar_mul(ident6_h[:, i, :], ident_bf, 0.5)
            with _maybe_repeat(tc, nc, repeat):
                # g FIRST on the SP queue so its transfer wins the DMA
                # engines over the x chunks
                g_bf = gpool.tile([128, 2 * NROT, 128], BF16, tag="g_bf")
                nc.sync.dma_start(out=g_bf, in_=g_d[:, :, :])
                junk = psJ.tile([128, 128], F32, tag="junk")
                for _ in range(6):
                    nc.tensor.matmul(junk, lhsT=warm, rhs=warm,
                                     start=True, stop=True)
                xchunks = []
                for b in range(BPC):
                    for c in range(NCH):
                        xt = xpool.tile([128, CW], F8, tag="xt")
                        nc.sync.dma_start(out=xt, in_=x[b, c])
                        xchunks.append((b, c, xt))

                # ---- expm deg-4 + chain ----
                # T4(G) = (I + G) + G2 @ tb,  tb = I/2 + G/6 + G2/24
                # m2 := mm(g, g) = -G2  (G antisymmetric);  tbneg := -tb
                #     = m2/24 - ihg,  ihg := I/2 + G/6
                # r_ps = mm(I,I) + mm(I,g) + mm(m2_sb, tbneg)
                # rt (= R0^T, T4(-G0)): tbneg_n = m2/24 - ihg_n,
                #     ihg_n := I/2 - G0/6;  rt_ps = mm(I,I) + mm(g0,I)
                #     + mm(m2_sb0, tbneg_n)
                # chain: p01 = mm(R1, rtT) = (R0R1)^T; Rall = mm(r01t, R2)
                ihg = gpool.tile([128, 2 * NROT, 128], BF16, tag="ihg")
                nc.vector.scalar_tensor_tensor(
                    ihg, g_bf, 1.0 / 6.0, ident6_h, A.mult, A.add
                )
                ihg_n = gpool.tile([128, BPC, 128], BF16, tag="ihg_n")
                for b in range(BPC):
                    nc.vector.scalar_tensor_tensor(
                        ihg_n[:, b, :], g_bf[:, b * NROT, :], -1.0 / 6.0,
                        ident6_h[:, 0, :], A.mult, A.add,
                    )
                m2_sb = gpool.tile([128, 2 * NROT, 128], BF16, tag="m2_sb")
                tbneg = gpool.tile([128, 2 * NROT, 128], BF16, tag="tbneg")
                tbneg_n = gpool.tile([128, BPC, 128], BF16, tag="tbneg_n")
                r_sb = gpool.tile([128, 2 * NROT, 128], BF16, tag="r_sb")
                rt_sb = gpool.tile([128, BPC, 128], BF16, tag="rt_sb")
                rm16 = [None, None]

                def expm_batch(b):
                    sl = slice(b * NROT, (b + 1) * NROT)
                    g_i = [g_bf[:, b * NROT + i, :] for i in range(NROT)]
                    m2 = psE.tile([128, NROT, 128], F32, tag="m2", name="m2")
                    for i in range(NROT):
                        nc.tensor.matmul(m2[:, i, :], lhsT=g_i[i], rhs=g_i[i],
                                         start=True, stop=True)
                    nc.scalar.copy(m2_sb[:, sl, :], m2)
                    nc.vector.scalar_tensor_tensor(
                        tbneg[:, sl, :], m2_sb[:, sl, :], 1.0 / 24.0,
                        ihg[:, sl, :], A.mult, A.subtract,
                    )
                    nc.vector.scalar_tensor_tensor(
                        tbneg_n[:, b, :], m2_sb[:, b * NROT, :], 1.0 / 24.0,
                        ihg_n[:, b, :], A.mult, A.subtract,
                    )
                    rp = psE.tile([128, NROT + 1, 128], F32, tag="rp", name="rp")
                    m2s = [m2_sb[:, b * NROT + i, :] for i in range(NROT)]
                    for i in range(NROT):
                        nc.tensor.matmul(rp[:, i, :], lhsT=ident_bf,
                                         rhs=ident_bf, start=True, stop=False)
                        nc.tensor.matmul(rp[:, i, :], lhsT=ident_bf,
                                         rhs=g_i[i], start=False, stop=False)
                        nc.tensor.matmul(rp[:, i, :], lhsT=m2s[i],
                                         rhs=tbneg[:, b * NROT + i, :],
                                         start=False, stop=True)
                    nc.tensor.matmul(rp[:, NROT, :], lhsT=ident_bf,
                                     rhs=ident_bf, start=True, stop=False)
                    nc.tensor.matmul(rp[:, NROT, :], lhsT=g_i[0], rhs=ident_bf,
                                     start=False, stop=False)
                    nc.tensor.matmul(rp[:, NROT, :], lhsT=m2s[0],
                                     rhs=tbneg_n[:, b, :],
                                     start=False, stop=True)
                    nc.scalar.copy(r_sb[:, sl, :], rp[:, 0:NROT, :])
                    nc.scalar.copy(rt_sb[:, b, :], rp[:, NROT, :])
                    ch = psE.tile([128, 128], F32, tag="ch", name="ch")
                    nc.tensor.matmul(
                        ch, lhsT=r_sb[:, b * NROT + 1, :], rhs=rt_sb[:, b, :],
                        start=True, stop=True,
                    )
                    r01t = chpool.tile([128, 128], BF16, tag="r01t")
                    nc.scalar.copy(r01t, ch)
                    ch2 = psE.tile([128, 128], F32, tag="ch", name="ch2")
                    nc.tensor.matmul(
                        ch2, lhsT=r01t, rhs=r_sb[:, b * NROT + 2, :],
                        start=True, stop=True,
                    )
                    rb = chpool.tile([128, 128], BF16, tag="rall")
                    # Rm = 16*Rall - 16*I, the stationary einsum operand
                    nc.vector.scalar_tensor_tensor(
                        rb, ch2, DSC, ident16, A.mult, A.subtract
                    )
                    rm16[b] = rb

                dq_sb = [None]

                def einsum_chunk(i):
                    b, c, xt = xchunks[i]
                    if c % 2 == 0:
                        dq_sb[0] = ypool.tile(
                            [128, 2, CW], F8, tag="dq_sb", name="dq_sb"
                        )
                    dsb = dq_sb[0]
                    for q2 in range(2):
                        yp = psY.tile([128, 2, 512], F32, tag="yp")
                        for q in range(2):
                            nc.tensor.matmul(
                                yp[:, q, :],
                                lhsT=rm16[b],
                                rhs=xt[:, 1024 * q2 + 512 * q : 1024 * q2 + 512 * (q + 1)],
                                start=True, stop=True,
                            )
                        dst = dsb[:, c % 2, 1024 * q2 : 1024 * (q2 + 1)]
                        if (2 * i + q2) % 2 == 0:
                            nc.scalar.activation(
                                dst, yp.rearrange("p a x -> p (a x)"), AF.Copy,
                                bias=0.0, scale=1.0,
                            )
                        else:
                            nc.vector.tensor_copy(
                                dst, yp.rearrange("p a x -> p (a x)")
                            )
                    # ramp filler: no-dep matmuls on the already-loaded
                    # stationary keep the PE p-state from resetting
                    for _ in range(2):
                        nc.tensor.matmul(junk[:, 0:16], lhsT=rm16[b],
                                         rhs=warm[:, 0:16], start=True,
                                         stop=True)
                    if c % 2 == 1:
                        q = nc.sync if (i // 2) % 2 == 0 else nc.scalar
                        q.dma_start(out=dq[b][:, c - 1 : c + 1, :], in_=dsb)

                expm_batch(0)
                einsum_chunk(0)
                einsum_chunk(1)
                expm_batch(1)
                for i in range(2, len(xchunks)):
                    einsum_chunk(i)
    _bench_io(nc, io_internal)
    return _split_sync_waits(_elide_ldweights(nc)) if split else nc


_CACHE = {}
_PREP = {}


def _get(name):
    if name not in _CACHE:
        _CACHE[name] = {
            "pool": build_pool, "params": build_params, "rot": build_rot
        }[name]()
    return _CACHE[name]


def _prep_weights(W1, b1, W2, b2):
    key = (float(np.asarray(W2).flat[0]), float(np.asarray(W2).flat[-1]),
           float(np.asarray(b2).flat[0]), float(np.asarray(b1).flat[0]))
    if _PREP.get("key") == key:
        return
    import ml_dtypes

    W1 = np.asarray(W1, np.float64)
    _PREP["w1t"] = np.ascontiguousarray(W1.T / S, dtype=np.float32)
    _PREP["b1q"] = np.ascontiguousarray(
        np.asarray(b1, np.float32).reshape(KT, 128).T
    )

    iu, ju = np.triu_indices(D, k=1)                  # 8128 strict-upper pairs
    _PREP["iu"], _PREP["ju"] = iu, ju
    V = np.asarray(W2, np.float64).reshape(NROT, D, D, H)
    WU = 0.5 * (V[:, iu, ju, :] - V[:, ju, iu, :])    # [r, 8128, k]
    WU = WU.reshape(NROT * NU, H)                     # rows = packed (r, u)
    # shard c takes packed rows [c*3048, (c+1)*3048), padded to 3072
    shards = []
    per = NROT * NU // NCORES                         # 3048
    for c in range(NCORES):
        blk = np.zeros((JPCU, H), np.float64)
        blk[:per] = WU[c * per : (c + 1) * per]
        sh = np.ascontiguousarray(blk.T * 64.0).astype(ml_dtypes.float8_e4m3fn)
        pm = sh.reshape(KT, 128, NPAN, NJ).transpose(2, 1, 0, 3)
        shards.append(np.ascontiguousarray(pm.reshape(NPAN, 128, KT * NJ)))
    _PREP["w2u"] = shards
    b2m = np.asarray(b2, np.float64).reshape(NROT, D, D)
    bg = 0.5 * (b2m - b2m.transpose(0, 2, 1))         # [r, i, col] skew bias
    _PREP["b2g"] = np.ascontiguousarray(bg, dtype=np.float32)
    _PREP["key"] = key


def _prep_x(x):
    import ml_dtypes

    x8 = np.asarray(x, np.float32).astype(ml_dtypes.float8_e4m3fn)
    v = x8.view(np.uint8).reshape(B, NCH, CW, D)
    # xq8[b, c, d, j] = x[b, 2048c + j, d]
    xq8 = np.ascontiguousarray(v.transpose(0, 1, 3, 2)).view(
        ml_dtypes.float8_e4m3fn
    )
    # xn8[b, c, sp, t, d] = x[b, 2048c + 128t + sp, d]
    xn8 = np.ascontiguousarray(
        v.reshape(B, NCH, TPC, 128, D).transpose(0, 1, 3, 2, 4)
    ).view(ml_dtypes.float8_e4m3fn)
    return xq8, xn8


def kernel(x, W1, b1, W2, b2):
    _prep_weights(W1, b1, W2, b2)
    xq8, xn8 = _prep_x(x)
    cores = list(range(NCORES))

    # ---- L1: pooled sums ----
    in1 = [{"xn8": xn8[c * BPC : (c + 1) * BPC]} for c in cores]
    r1 = run_bass_kernel_spmd(_get("pool"), in1, core_ids=cores)
    pall = np.concatenate(
        [np.asarray(r1.results[c]["pooled"]) for c in cores], axis=0
    )  # [B, D] sums

    # ---- L2: packed skew-generator entries (device MLP + W2u matmuls) ----
    sin = np.ascontiguousarray(
        np.concatenate(
            [pall.T.astype(np.float32), _PREP["w1t"], _PREP["b1q"]], axis=1
        )
    )
    in2 = [{"sin": sin, "w2u": _PREP["w2u"][c]} for c in cores]
    r2 = run_bass_kernel_spmd(_get("params"), in2, core_ids=cores)

    # ---- host: scatter upper entries -> full G (mirror + bias) ----
    per = NROT * NU // NCORES
    up = np.concatenate(
        [np.asarray(r2.results[c]["params"], dtype=np.float32)[:, :per]
         for c in cores], axis=1,
    ).reshape(B, NROT, NU)
    iu, ju = _PREP["iu"], _PREP["ju"]
    G = np.zeros((B, NROT, D, D), dtype=np.float32)
    G[:, :, iu, ju] = up
    G[:, :, ju, iu] = -up
    G += _PREP["b2g"][None]
    import ml_dtypes

    gs = []
    for c in cores:
        gb = G[c * BPC : (c + 1) * BPC].transpose(2, 0, 1, 3)  # [i, b, r, col]
        gs.append(np.ascontiguousarray(
            gb.reshape(D, 2 * NROT, D).astype(ml_dtypes.bfloat16)))

    # ---- L3: expm + chain + delta einsum ----
    in3 = [{"xq8": xq8[c * BPC : (c + 1) * BPC], "g": gs[c]} for c in cores]
    r3 = run_bass_kernel_spmd(_get("rot"), in3, core_ids=cores)
    dall = np.concatenate(
        [np.asarray(r3.results[c]["dq"]) for c in cores], axis=0
    )  # [B, D, NCH, CW] fp8 (x16)
    delta = np.asarray(dall, dtype=np.float32) * (1.0 / DSC)
    delta = delta.transpose(0, 2, 3, 1).reshape(B, S, D)
    return np.asarray(x, np.float32) + delta
